# revision 1
# baseline (speedup 1.0000x reference)
"""Trainium2 Bass kernel for nn_CAM01 (topk_masking).

Pipeline per image (one image per NeuronCore, pure data parallel over B=8):
  conv0 1x1 (512->30, fp32 matmul) + folded BN bias/scale + ReLU
  conv1 1x1 (30->17, fp32 matmul) + bias -> logits (output "out")
  per-pixel argmax over 17 classes:
      DVE 32x32 block-transpose of logits -> free-dim reduce_max ->
      is_ge mask (exact 0/1) -> block-transpose back (bf16)
  res[k*30+j, pix] = h[j, pix] * mask[k, pix]:
      mask replicated 17->510 rows and h replicated 30->510 rows via
      EXACT bf16 0/1-matrix matmuls, multiplied on DVE, DMA'd out.

Layout: channels on partitions, pixels on the free dim everywhere; the
HW x/res/out tensors are [C, H*W] per image.
"""

import sys

sys.path.insert(0, "/opt/trn_rl_repo")

import numpy as np
import ml_dtypes

import concourse.bass as bass
import concourse.mybir as mybir
from concourse import bacc
from concourse.tile import TileContext
from concourse.bass_utils import run_bass_kernel_spmd

F32 = mybir.dt.float32
BF16 = mybir.dt.bfloat16

B, CIN, H, W = 8, 512, 128, 128
PIX = H * W                     # 16384
CMID = 30                       # conv0 out channels
NCLS = 17                       # classes
CRES = NCLS * CMID              # 510
NT = 512                        # pixels per tile
NTILES = PIX // NT              # 32
BN_EPS = 1e-5

# class-aligned chunks of the 510 res channels
CHUNKS = [(0, 120), (120, 120), (240, 120), (360, 120), (480, 30)]

_COMPILED = None  # (nc, name lists) built once per process


def _build():
    nc = bacc.Bacc()

    x_d = nc.declare_dram_parameter("xp", [CIN, PIX], F32, isOutput=False)
    w0_d = nc.declare_dram_parameter("w0t", [CIN, CMID], F32, isOutput=False)
    b0_d = nc.declare_dram_parameter("b0p", [CMID, 1], F32, isOutput=False)
    w1_d = nc.declare_dram_parameter("w1t", [CMID, NCLS], F32, isOutput=False)
    b1_d = nc.declare_dram_parameter("b1p", [NCLS, 1], F32, isOutput=False)
    sr_d = nc.declare_dram_parameter("srep", [NCLS, CRES], BF16, isOutput=False)
    rr_d = nc.declare_dram_parameter("rrep", [CMID, CRES], BF16, isOutput=False)

    res_d = nc.declare_dram_parameter("res", [CRES, PIX], F32, isOutput=True)
    out_d = nc.declare_dram_parameter("out", [NCLS, PIX], F32, isOutput=True)

    with TileContext(nc) as tc:
        with (
            tc.tile_pool(name="consts", bufs=1) as cp,
            tc.tile_pool(name="xin", bufs=3) as xp,
            tc.tile_pool(name="work", bufs=3) as wk,
            tc.tile_pool(name="mrsb", bufs=3) as mp,
            tc.tile_pool(name="resp", bufs=6) as rp,
            tc.tile_pool(name="psum", bufs=2, space="PSUM") as ps,
        ):
            w0_t = cp.tile([128, 4, CMID], F32, tag="w0")
            nc.sync.dma_start(out=w0_t, in_=w0_d.rearrange("(c p) m -> p c m", p=128))
            b0_t = cp.tile([CMID, 1], F32, tag="b0")
            nc.sync.dma_start(out=b0_t, in_=b0_d[:, :])
            w1_t = cp.tile([CMID, NCLS], F32, tag="w1")
            nc.sync.dma_start(out=w1_t, in_=w1_d[:, :])
            b1_t = cp.tile([NCLS, 1], F32, tag="b1")
            nc.sync.dma_start(out=b1_t, in_=b1_d[:, :])
            sr_t = cp.tile([NCLS, CRES], BF16, tag="sr")
            nc.sync.dma_start(out=sr_t, in_=sr_d[:, :])
            rr_t = cp.tile([CMID, CRES], BF16, tag="rr")
            nc.sync.dma_start(out=rr_t, in_=rr_d[:, :])

            for t in range(NTILES):
                sl = slice(t * NT, (t + 1) * NT)

                x_t = xp.tile([128, 4, NT], F32, tag="x")
                nc.sync.dma_start(
                    out=x_t, in_=x_d[:, sl].rearrange("(c p) n -> p c n", p=128)
                )

                # conv0: psum_h[30, NT] = w0'.T @ x  (fp32, K=512 in 4 chunks)
                p_h = ps.tile([CMID, NT], F32, tag="ph")
                for i in range(4):
                    nc.tensor.matmul(p_h, lhsT=w0_t[:, i, :], rhs=x_t[:, i, :],
                                     start=(i == 0), stop=(i == 3))

                # h = relu(psum_h + b0')  (f32 for conv1, bf16 for replication)
                h_sb = wk.tile([CMID, NT], F32, tag="h")
                nc.scalar.activation(h_sb, p_h, mybir.ActivationFunctionType.Relu,
                                     bias=b0_t, scale=1.0)
                h16 = wk.tile([CMID, NT], BF16, tag="h16")
                nc.scalar.activation(h16, p_h, mybir.ActivationFunctionType.Relu,
                                     bias=b0_t, scale=1.0)

                # conv1: logits[17, NT] = w1.T @ h + b1
                p_o = ps.tile([NCLS, NT], F32, tag="po")
                nc.tensor.matmul(p_o, lhsT=w1_t, rhs=h_sb, start=True, stop=True)
                lg32 = wk.tile([32, NT], F32, tag="lg32")
                nc.scalar.activation(lg32[0:NCLS, :], p_o,
                                     mybir.ActivationFunctionType.Identity,
                                     bias=b1_t, scale=1.0)
                nc.sync.dma_start(out=out_d[:, sl], in_=lg32[0:NCLS, :])

                # argmax mask via DVE 32x32 block transpose
                t32 = wk.tile([32, NT], F32, tag="t32")
                nc.vector.transpose(t32, lg32)
                tv = t32.rearrange("p (b c) -> p b c", c=32)[:, :, 0:NCLS]
                mxv = wk.tile([32, NT // 32], F32, tag="mxv")
                nc.vector.tensor_reduce(mxv, tv, axis=mybir.AxisListType.X,
                                        op=mybir.AluOpType.max)
                mk_t = wk.tile([32, NT], BF16, tag="mkt")
                mk_tv = mk_t.rearrange("p (b c) -> p b c", c=32)[:, :, 0:NCLS]
                nc.vector.tensor_tensor(
                    out=mk_tv, in0=tv,
                    in1=mxv.unsqueeze(2).to_broadcast([32, NT // 32, NCLS]),
                    op=mybir.AluOpType.is_ge)
                mk16 = wk.tile([32, NT], BF16, tag="mk16")
                nc.vector.transpose(mk16, mk_t)

                # res chunks: replicate mask (exact) + h16, multiply, store
                for c0, cs in CHUNKS:
                    p_m = ps.tile([cs, NT], F32, tag="mrep", name=f"pm{t}_{c0}")
                    nc.tensor.matmul(p_m, lhsT=sr_t[:, c0:c0 + cs],
                                     rhs=mk16[0:NCLS, :], start=True, stop=True)
                    m_sb = mp.tile([cs, NT], F32, tag="msb", name=f"ms{t}_{c0}")
                    nc.scalar.copy(m_sb, p_m)

                    p_r = ps.tile([cs, NT], F32, tag="hrep", name=f"pr{t}_{c0}")
                    nc.tensor.matmul(p_r, lhsT=rr_t[:, c0:c0 + cs],
                                     rhs=h16, start=True, stop=True)
                    r_sb = rp.tile([cs, NT], F32, tag="res", name=f"rs{t}_{c0}")
                    nc.vector.tensor_tensor(out=r_sb, in0=p_r, in1=m_sb,
                                            op=mybir.AluOpType.mult)
                    nc.sync.dma_start(out=res_d[c0:c0 + cs, sl], in_=r_sb)

    nc.compile()
    return nc


def _get_compiled():
    global _COMPILED
    if _COMPILED is None:
        _COMPILED = _build()
    return _COMPILED


def kernel(x, w0, b0, gamma, beta, mean, var, w1, b1):
    x = np.asarray(x, dtype=np.float32)
    w0 = np.asarray(w0, dtype=np.float32)
    b0 = np.asarray(b0, dtype=np.float32)
    gamma = np.asarray(gamma, dtype=np.float32)
    beta = np.asarray(beta, dtype=np.float32)
    mean = np.asarray(mean, dtype=np.float32)
    var = np.asarray(var, dtype=np.float32)
    w1 = np.asarray(w1, dtype=np.float32)
    b1 = np.asarray(b1, dtype=np.float32)

    # fold BN into conv0 weights/bias (f32 host math)
    scale = gamma / np.sqrt(var + np.float32(BN_EPS))
    w0t = np.ascontiguousarray((w0 * scale[:, None]).T.astype(np.float32))  # [512, 30]
    b0p = ((b0 - mean) * scale + beta).astype(np.float32).reshape(CMID, 1)
    w1t = np.ascontiguousarray(w1.T.astype(np.float32))                      # [30, 17]
    b1p = b1.astype(np.float32).reshape(NCLS, 1)

    # replication matrices (exact 0/1 in bf16)
    cc = np.arange(CRES)
    srep = np.zeros((NCLS, CRES), dtype=ml_dtypes.bfloat16)
    srep[cc // CMID, cc] = 1
    rrep = np.zeros((CMID, CRES), dtype=ml_dtypes.bfloat16)
    rrep[cc % CMID, cc] = 1

    nc = _get_compiled()
    in_maps = []
    for i in range(B):
        in_maps.append({
            "xp": np.ascontiguousarray(x[i].reshape(CIN, PIX)),
            "w0t": w0t, "b0p": b0p, "w1t": w1t, "b1p": b1p,
            "srep": srep, "rrep": rrep,
        })
    results = run_bass_kernel_spmd(nc, in_maps, core_ids=list(range(B))).results

    res = np.empty((B, CRES, H, W), dtype=np.float32)
    out = np.empty((B, NCLS, H, W), dtype=np.float32)
    for i in range(B):
        res[i] = results[i]["res"].reshape(CRES, H, W)
        out[i] = results[i]["out"].reshape(NCLS, H, W)
    return res, out


# revision 2
# speedup vs baseline: 25494.6219x; 25494.6219x over previous
"""Trainium2 Bass kernel for nn_CAM01 (topk_masking).

Pipeline per image (one image per NeuronCore, pure data parallel over B=8):
  conv0 1x1 (512->30, fp32 matmul) + folded BN bias/scale + ReLU
  conv1 1x1 (30->17, fp32 matmul) + bias -> logits (output "out")
  per-pixel argmax over 17 classes:
      DVE 32x32 block-transpose of logits -> free-dim reduce_max ->
      is_ge mask (exact 0/1) -> block-transpose back (bf16)
  res[k*30+j, pix] = h[j, pix] * mask[k, pix]:
      mask replicated 17->510 rows and h replicated 30->510 rows via
      EXACT bf16 0/1-matrix matmuls, multiplied on DVE, DMA'd out.

Layout: channels on partitions, pixels on the free dim everywhere; the
HW x/res/out tensors are [C, H*W] per image.
"""

import sys

sys.path.insert(0, "/opt/trn_rl_repo")

import numpy as np
import ml_dtypes

import concourse.bass as bass
import concourse.mybir as mybir
from concourse import bacc
from concourse.tile import TileContext
from concourse.bass_utils import run_bass_kernel_spmd

F32 = mybir.dt.float32
BF16 = mybir.dt.bfloat16

B, CIN, H, W = 8, 512, 128, 128
PIX = H * W                     # 16384
CMID = 30                       # conv0 out channels
NCLS = 17                       # classes
CRES = NCLS * CMID              # 510
NT = 512                        # pixels per tile
NTILES = PIX // NT              # 32
BN_EPS = 1e-5

# class-aligned chunks of the 510 res channels
CHUNKS = [(0, 120), (120, 120), (240, 120), (360, 120), (480, 30)]

_COMPILED = None  # (nc, name lists) built once per process


def _build(reps: int = 1):
    nc = bacc.Bacc()

    x_d = nc.declare_dram_parameter("xp", [CIN, PIX], F32, isOutput=False)
    w0_d = nc.declare_dram_parameter("w0t", [CIN, CMID], F32, isOutput=False)
    b0_d = nc.declare_dram_parameter("b0p", [CMID, 1], F32, isOutput=False)
    w1_d = nc.declare_dram_parameter("w1t", [CMID, NCLS], F32, isOutput=False)
    b1_d = nc.declare_dram_parameter("b1p", [NCLS, 1], F32, isOutput=False)
    sr_d = nc.declare_dram_parameter("srep", [NCLS, CRES], BF16, isOutput=False)
    rr_d = nc.declare_dram_parameter("rrep", [CMID, CRES], BF16, isOutput=False)

    res_d = nc.declare_dram_parameter("res", [CRES, PIX], F32, isOutput=True)
    out_d = nc.declare_dram_parameter("out", [NCLS, PIX], F32, isOutput=True)

    with TileContext(nc) as tc:
        with (
            tc.tile_pool(name="consts", bufs=1) as cp,
            tc.tile_pool(name="xin", bufs=3) as xp,
            tc.tile_pool(name="work", bufs=3) as wk,
            tc.tile_pool(name="mrsb", bufs=3) as mp,
            tc.tile_pool(name="resp", bufs=6) as rp,
            tc.tile_pool(name="psum", bufs=2, space="PSUM") as ps,
        ):
            w0_t = cp.tile([128, 4, CMID], F32, tag="w0")
            nc.sync.dma_start(out=w0_t, in_=w0_d.rearrange("(c p) m -> p c m", p=128))
            b0_t = cp.tile([CMID, 1], F32, tag="b0")
            nc.sync.dma_start(out=b0_t, in_=b0_d[:, :])
            w1_t = cp.tile([CMID, NCLS], F32, tag="w1")
            nc.sync.dma_start(out=w1_t, in_=w1_d[:, :])
            b1_t = cp.tile([NCLS, 1], F32, tag="b1")
            nc.sync.dma_start(out=b1_t, in_=b1_d[:, :])
            sr_t = cp.tile([NCLS, CRES], BF16, tag="sr")
            nc.sync.dma_start(out=sr_t, in_=sr_d[:, :])
            rr_t = cp.tile([CMID, CRES], BF16, tag="rr")
            nc.sync.dma_start(out=rr_t, in_=rr_d[:, :])

            def tile_body(t):
                sl = slice(t * NT, (t + 1) * NT)

                x_t = xp.tile([128, 4, NT], F32, tag="x")
                nc.sync.dma_start(
                    out=x_t, in_=x_d[:, sl].rearrange("(c p) n -> p c n", p=128)
                )

                # conv0: psum_h[30, NT] = w0'.T @ x  (fp32, K=512 in 4 chunks)
                p_h = ps.tile([CMID, NT], F32, tag="ph")
                for i in range(4):
                    nc.tensor.matmul(p_h, lhsT=w0_t[:, i, :], rhs=x_t[:, i, :],
                                     start=(i == 0), stop=(i == 3))

                # h = relu(psum_h + b0')  (f32 for conv1, bf16 for replication)
                h_sb = wk.tile([CMID, NT], F32, tag="h")
                nc.scalar.activation(h_sb, p_h, mybir.ActivationFunctionType.Relu,
                                     bias=b0_t, scale=1.0)
                h16 = wk.tile([CMID, NT], BF16, tag="h16")
                nc.scalar.activation(h16, p_h, mybir.ActivationFunctionType.Relu,
                                     bias=b0_t, scale=1.0)

                # conv1: logits[17, NT] = w1.T @ h + b1
                p_o = ps.tile([NCLS, NT], F32, tag="po")
                nc.tensor.matmul(p_o, lhsT=w1_t, rhs=h_sb, start=True, stop=True)
                lg32 = wk.tile([32, NT], F32, tag="lg32")
                nc.scalar.activation(lg32[0:NCLS, :], p_o,
                                     mybir.ActivationFunctionType.Identity,
                                     bias=b1_t, scale=1.0)
                nc.sync.dma_start(out=out_d[:, sl], in_=lg32[0:NCLS, :])

                # argmax mask via DVE 32x32 block transpose
                t32 = wk.tile([32, NT], F32, tag="t32")
                nc.vector.transpose(t32, lg32)
                tv = t32.rearrange("p (b c) -> p b c", c=32)[:, :, 0:NCLS]
                mxv = wk.tile([32, NT // 32], F32, tag="mxv")
                nc.vector.tensor_reduce(mxv, tv, axis=mybir.AxisListType.X,
                                        op=mybir.AluOpType.max)
                mk_t = wk.tile([32, NT], BF16, tag="mkt")
                mk_tv = mk_t.rearrange("p (b c) -> p b c", c=32)[:, :, 0:NCLS]
                nc.vector.tensor_tensor(
                    out=mk_tv, in0=tv,
                    in1=mxv.unsqueeze(2).to_broadcast([32, NT // 32, NCLS]),
                    op=mybir.AluOpType.is_ge)
                mk16 = wk.tile([32, NT], BF16, tag="mk16")
                nc.vector.transpose(mk16, mk_t)

                # res chunks: replicate mask (exact) + h16, multiply, store
                for c0, cs in CHUNKS:
                    p_m = ps.tile([cs, NT], F32, tag="mrep", name=f"pm{t}_{c0}")
                    nc.tensor.matmul(p_m, lhsT=sr_t[:, c0:c0 + cs],
                                     rhs=mk16[0:NCLS, :], start=True, stop=True)
                    m_sb = mp.tile([cs, NT], F32, tag="msb", name=f"ms{t}_{c0}")
                    nc.scalar.copy(m_sb, p_m)

                    p_r = ps.tile([cs, NT], F32, tag="hrep", name=f"pr{t}_{c0}")
                    nc.tensor.matmul(p_r, lhsT=rr_t[:, c0:c0 + cs],
                                     rhs=h16, start=True, stop=True)
                    r_sb = rp.tile([cs, NT], F32, tag="res", name=f"rs{t}_{c0}")
                    nc.vector.tensor_tensor(out=r_sb, in0=p_r, in1=m_sb,
                                            op=mybir.AluOpType.mult)
                    nc.sync.dma_start(out=res_d[c0:c0 + cs, sl], in_=r_sb)

            if reps == 1:
                for t in range(NTILES):
                    tile_body(t)
            else:
                with tc.For_i(0, reps, 1):
                    for t in range(NTILES):
                        tile_body(t)

    nc.compile()
    return nc


def _get_compiled():
    global _COMPILED
    if _COMPILED is None:
        _COMPILED = _build()
    return _COMPILED


def kernel(x, w0, b0, gamma, beta, mean, var, w1, b1):
    x = np.asarray(x, dtype=np.float32)
    w0 = np.asarray(w0, dtype=np.float32)
    b0 = np.asarray(b0, dtype=np.float32)
    gamma = np.asarray(gamma, dtype=np.float32)
    beta = np.asarray(beta, dtype=np.float32)
    mean = np.asarray(mean, dtype=np.float32)
    var = np.asarray(var, dtype=np.float32)
    w1 = np.asarray(w1, dtype=np.float32)
    b1 = np.asarray(b1, dtype=np.float32)

    # fold BN into conv0 weights/bias (f32 host math)
    scale = gamma / np.sqrt(var + np.float32(BN_EPS))
    w0t = np.ascontiguousarray((w0 * scale[:, None]).T.astype(np.float32))  # [512, 30]
    b0p = ((b0 - mean) * scale + beta).astype(np.float32).reshape(CMID, 1)
    w1t = np.ascontiguousarray(w1.T.astype(np.float32))                      # [30, 17]
    b1p = b1.astype(np.float32).reshape(NCLS, 1)

    # replication matrices (exact 0/1 in bf16)
    cc = np.arange(CRES)
    srep = np.zeros((NCLS, CRES), dtype=ml_dtypes.bfloat16)
    srep[cc // CMID, cc] = 1
    rrep = np.zeros((CMID, CRES), dtype=ml_dtypes.bfloat16)
    rrep[cc % CMID, cc] = 1

    nc = _get_compiled()
    in_maps = []
    for i in range(B):
        in_maps.append({
            "xp": np.ascontiguousarray(x[i].reshape(CIN, PIX)),
            "w0t": w0t, "b0p": b0p, "w1t": w1t, "b1p": b1p,
            "srep": srep, "rrep": rrep,
        })
    results = run_bass_kernel_spmd(nc, in_maps, core_ids=list(range(B))).results

    res = np.empty((B, CRES, H, W), dtype=np.float32)
    out = np.empty((B, NCLS, H, W), dtype=np.float32)
    for i in range(B):
        res[i] = results[i]["res"].reshape(CRES, H, W)
        out[i] = results[i]["out"].reshape(NCLS, H, W)
    return res, out


# revision 4
# speedup vs baseline: 26999.3192x; 1.0590x over previous
"""Trainium2 Bass kernel for nn_CAM01 (topk_masking).

Pipeline per image (one image per NeuronCore, pure data parallel over B=8):
  conv0 1x1 (512->30, fp32 matmul) + folded BN bias/scale + ReLU
  conv1 1x1 (30->17, fp32 matmul) + bias -> logits (output "out")
  per-pixel argmax over 17 classes:
      DVE 32x32 block-transpose of logits -> free-dim reduce_max ->
      is_ge mask (exact 0/1) -> block-transpose back (bf16)
  res[k*30+j, pix] = h[j, pix] * mask[k, pix]:
      mask replicated 17->510 rows and h replicated 30->510 rows via
      EXACT bf16 0/1-matrix matmuls, multiplied on DVE, DMA'd out.

Layout: channels on partitions, pixels on the free dim everywhere; the
HW x/res/out tensors are [C, H*W] per image.
"""

import sys

sys.path.insert(0, "/opt/trn_rl_repo")

import numpy as np
import ml_dtypes

import concourse.bass as bass
import concourse.mybir as mybir
from concourse import bacc
from concourse.tile import TileContext
from concourse.bass_utils import run_bass_kernel_spmd

F32 = mybir.dt.float32
BF16 = mybir.dt.bfloat16

B, CIN, H, W = 8, 512, 128, 128
PIX = H * W                     # 16384
CMID = 30                       # conv0 out channels
NCLS = 17                       # classes
CRES = NCLS * CMID              # 510
NT = 2048                       # pixels per DMA superblock
NTILES = PIX // NT              # 8
NS = 512                        # pixels per compute sub-tile (PSUM-sized)
NSUB = NT // NS                 # 4
BN_EPS = 1e-5

# class-aligned chunks of the 510 res channels
CHUNKS = [(0, 120), (120, 120), (240, 120), (360, 120), (480, 30)]

_COMPILED = None  # (nc, name lists) built once per process


def _build(reps: int = 1):
    nc = bacc.Bacc()

    x_d = nc.declare_dram_parameter("xp", [CIN, PIX], F32, isOutput=False)
    w0_d = nc.declare_dram_parameter("w0t", [CIN, CMID], F32, isOutput=False)
    b0_d = nc.declare_dram_parameter("b0p", [CMID, 1], F32, isOutput=False)
    w1_d = nc.declare_dram_parameter("w1t", [CMID, NCLS], F32, isOutput=False)
    b1_d = nc.declare_dram_parameter("b1p", [NCLS, 1], F32, isOutput=False)
    sr_d = nc.declare_dram_parameter("srep", [NCLS, CRES], BF16, isOutput=False)
    rr_d = nc.declare_dram_parameter("rrep", [CMID, CRES], BF16, isOutput=False)

    res_d = nc.declare_dram_parameter("res", [CRES, PIX], F32, isOutput=True)
    out_d = nc.declare_dram_parameter("out", [NCLS, PIX], F32, isOutput=True)

    with TileContext(nc) as tc:
        with (
            tc.tile_pool(name="consts", bufs=1) as cp,
            tc.tile_pool(name="xin", bufs=2) as xp,
            tc.tile_pool(name="work", bufs=3) as wk,
            tc.tile_pool(name="mrsb", bufs=3) as mp,
            tc.tile_pool(name="resp", bufs=2) as rp,
            tc.tile_pool(name="psum", bufs=2, space="PSUM") as ps,
        ):
            w0_t = cp.tile([128, 4, CMID], F32, tag="w0")
            nc.sync.dma_start(out=w0_t, in_=w0_d.rearrange("(c p) m -> p c m", p=128))
            b0_t = cp.tile([CMID, 1], F32, tag="b0")
            nc.sync.dma_start(out=b0_t, in_=b0_d[:, :])
            w1_t = cp.tile([CMID, NCLS], F32, tag="w1")
            nc.sync.dma_start(out=w1_t, in_=w1_d[:, :])
            b1_t = cp.tile([NCLS, 1], F32, tag="b1")
            nc.sync.dma_start(out=b1_t, in_=b1_d[:, :])
            sr_t = cp.tile([NCLS, CRES], BF16, tag="sr")
            nc.sync.dma_start(out=sr_t, in_=sr_d[:, :])
            rr_t = cp.tile([CMID, CRES], BF16, tag="rr")
            nc.sync.dma_start(out=rr_t, in_=rr_d[:, :])

            def tile_body(t):
                sl = slice(t * NT, (t + 1) * NT)

                # one 4 MiB input DMA per superblock
                x_t = xp.tile([128, 4, NT], F32, tag="x")
                nc.sync.dma_start(
                    out=x_t, in_=x_d[:, sl].rearrange("(c p) n -> p c n", p=128)
                )
                lg32 = wk.tile([32, NT], F32, tag="lg32")
                res_big = rp.tile([120, 4, NT], F32, tag="resb")
                res_sm = rp.tile([CMID, NT], F32, tag="ress")

                for s in range(NSUB):
                    ss = slice(s * NS, (s + 1) * NS)

                    # conv0: psum_h[30, NS] = w0'.T @ x (fp32, K=512 in 4 chunks)
                    p_h = ps.tile([CMID, NS], F32, tag="ph", name=f"ph{t}_{s}")
                    for i in range(4):
                        nc.tensor.matmul(p_h, lhsT=w0_t[:, i, :], rhs=x_t[:, i, ss],
                                         start=(i == 0), stop=(i == 3))

                    # h = relu(psum_h + b0') (f32 for conv1, bf16 for replication)
                    h_sb = wk.tile([CMID, NS], F32, tag="h", name=f"h{t}_{s}")
                    nc.scalar.activation(h_sb, p_h, mybir.ActivationFunctionType.Relu,
                                         bias=b0_t, scale=1.0)
                    h16 = wk.tile([CMID, NS], BF16, tag="h16", name=f"h16_{t}_{s}")
                    nc.scalar.activation(h16, p_h, mybir.ActivationFunctionType.Relu,
                                         bias=b0_t, scale=1.0)

                    # conv1: logits[17, NS] = w1.T @ h + b1
                    p_o = ps.tile([NCLS, NS], F32, tag="po", name=f"po{t}_{s}")
                    nc.tensor.matmul(p_o, lhsT=w1_t, rhs=h_sb, start=True, stop=True)
                    nc.scalar.activation(lg32[0:NCLS, ss], p_o,
                                         mybir.ActivationFunctionType.Identity,
                                         bias=b1_t, scale=1.0)

                    # argmax mask via DVE 32x32 block transpose
                    t32 = wk.tile([32, NS], F32, tag="t32", name=f"t32_{t}_{s}")
                    nc.vector.transpose(t32, lg32[:, ss])
                    tv = t32.rearrange("p (b c) -> p b c", c=32)[:, :, 0:NCLS]
                    mxv = wk.tile([32, NS // 32], F32, tag="mxv", name=f"mx{t}_{s}")
                    nc.vector.tensor_reduce(mxv, tv, axis=mybir.AxisListType.X,
                                            op=mybir.AluOpType.max)
                    mk_t = wk.tile([32, NS], BF16, tag="mkt", name=f"mkt{t}_{s}")
                    mk_tv = mk_t.rearrange("p (b c) -> p b c", c=32)[:, :, 0:NCLS]
                    nc.vector.tensor_tensor(
                        out=mk_tv, in0=tv,
                        in1=mxv.unsqueeze(2).to_broadcast([32, NS // 32, NCLS]),
                        op=mybir.AluOpType.is_ge)
                    mk16 = wk.tile([32, NS], BF16, tag="mk16", name=f"mk16_{t}_{s}")
                    nc.vector.transpose(mk16, mk_t)

                    # res chunks: replicate mask (exact) + h16, multiply
                    for ci, (c0, cs) in enumerate(CHUNKS):
                        p_m = ps.tile([cs, NS], F32, tag="mrep", name=f"pm{t}_{s}_{c0}")
                        nc.tensor.matmul(p_m, lhsT=sr_t[:, c0:c0 + cs],
                                         rhs=mk16[0:NCLS, :], start=True, stop=True)
                        m_sb = mp.tile([cs, NS], F32, tag="msb", name=f"ms{t}_{s}_{c0}")
                        nc.scalar.copy(m_sb, p_m)

                        p_r = ps.tile([cs, NS], F32, tag="hrep", name=f"pr{t}_{s}_{c0}")
                        nc.tensor.matmul(p_r, lhsT=rr_t[:, c0:c0 + cs],
                                         rhs=h16, start=True, stop=True)
                        dst = (res_big[:, ci, ss] if ci < 4
                               else res_sm[:, ss])
                        nc.vector.tensor_tensor(out=dst, in0=p_r, in1=m_sb,
                                                op=mybir.AluOpType.mult)

                # 4 output DMAs per superblock
                nc.sync.dma_start(out=out_d[:, sl], in_=lg32[0:NCLS, :])
                nc.sync.dma_start(
                    out=res_d[0:480, :].rearrange("(c p) n -> p c n", p=120)[:, :, sl],
                    in_=res_big)
                nc.sync.dma_start(out=res_d[480:510, sl], in_=res_sm)

            if reps == 1:
                for t in range(NTILES):
                    tile_body(t)
            else:
                with tc.For_i(0, reps, 1):
                    for t in range(NTILES):
                        tile_body(t)

    nc.compile()
    return nc


def _get_compiled():
    global _COMPILED
    if _COMPILED is None:
        _COMPILED = _build()
    return _COMPILED


def kernel(x, w0, b0, gamma, beta, mean, var, w1, b1):
    x = np.asarray(x, dtype=np.float32)
    w0 = np.asarray(w0, dtype=np.float32)
    b0 = np.asarray(b0, dtype=np.float32)
    gamma = np.asarray(gamma, dtype=np.float32)
    beta = np.asarray(beta, dtype=np.float32)
    mean = np.asarray(mean, dtype=np.float32)
    var = np.asarray(var, dtype=np.float32)
    w1 = np.asarray(w1, dtype=np.float32)
    b1 = np.asarray(b1, dtype=np.float32)

    # fold BN into conv0 weights/bias (f32 host math)
    scale = gamma / np.sqrt(var + np.float32(BN_EPS))
    w0t = np.ascontiguousarray((w0 * scale[:, None]).T.astype(np.float32))  # [512, 30]
    b0p = ((b0 - mean) * scale + beta).astype(np.float32).reshape(CMID, 1)
    w1t = np.ascontiguousarray(w1.T.astype(np.float32))                      # [30, 17]
    b1p = b1.astype(np.float32).reshape(NCLS, 1)

    # replication matrices (exact 0/1 in bf16)
    cc = np.arange(CRES)
    srep = np.zeros((NCLS, CRES), dtype=ml_dtypes.bfloat16)
    srep[cc // CMID, cc] = 1
    rrep = np.zeros((CMID, CRES), dtype=ml_dtypes.bfloat16)
    rrep[cc % CMID, cc] = 1

    nc = _get_compiled()
    in_maps = []
    for i in range(B):
        in_maps.append({
            "xp": np.ascontiguousarray(x[i].reshape(CIN, PIX)),
            "w0t": w0t, "b0p": b0p, "w1t": w1t, "b1p": b1p,
            "srep": srep, "rrep": rrep,
        })
    results = run_bass_kernel_spmd(nc, in_maps, core_ids=list(range(B))).results

    res = np.empty((B, CRES, H, W), dtype=np.float32)
    out = np.empty((B, NCLS, H, W), dtype=np.float32)
    for i in range(B):
        res[i] = results[i]["res"].reshape(CRES, H, W)
        out[i] = results[i]["out"].reshape(NCLS, H, W)
    return res, out
